# revision 17
# baseline (speedup 1.0000x reference)
"""Trainium2 Bass kernel for nn_Attention_16612933500996.

Full-input contract: kernel(**inputs) takes the unsharded inputs and returns
the full output. Internally shards across 8 NeuronCores: core i handles
batch b = i//2 and query-half w = i%2 (1024 of 2048 tokens). No collectives:
each core recomputes K/V for its whole batch (x rows are rotated host-side so
each core's query tokens are always rows 0..1023 — softmax over keys is
permutation invariant).

Per-core pipeline (all matmuls bf16 -> f32 PSUM):
  0. PE-transpose x [t,d] -> xT [d,t] (bf16)
  1. QKV projection: qT/kT produced transposed ([head*64+c, t]); V produced
     natural ([t, head-major cols]) with a fused ones-column per head so the
     attention U-matmul also yields the softmax denominator row.
  2. Attention per head: scoresT[m,w] = kT.T @ qT; exp via ACT (scores are
     ~±0.8 so no max-subtraction needed); U[65,w] = v_aug.T @ exp accumulated
     over key tiles (row 64 = sum of exps); normalize U/S with a PE-broadcast
     reciprocal; result nvT[e,w].
  3. Output projection (per-head K=64 accumulation) + bias + swish + residual
     + layernorm, DMA out.
"""

import sys

sys.path.insert(0, "/opt/trn_rl_repo")

import numpy as np

import concourse.bass as bass
import concourse.tile as tile
from concourse import mybir
from concourse.bass_utils import run_bass_kernel_spmd

AF = mybir.ActivationFunctionType
ALU = mybir.AluOpType
F32 = mybir.dt.float32
F32R = mybir.dt.float32r
BF16 = mybir.dt.bfloat16

B, L, D = 4, 2048, 1024
H, HD = 16, 64
WQ = 1024          # query tokens per core
N_CORES = 8
SCALE = 1.0 / float(np.sqrt(np.float32(L)))
LN_EPS = 1e-5


def _patch_tile_drain():
    """walrus in this container only accepts 1 sem wait on the TPB_CTRL drain;
    split the TileContext tail-drain waits across multiple drain instructions."""
    if getattr(tile.TileContext, "_drain_patched", False):
        return
    from concourse.tile import ScopedClock

    def _drain_and_barrier(self, tick_clock, wait_clock):
        nc = self.nc
        drain_inst = nc.sync.drain()
        wait_clock.add_sem_waits(
            drain_inst.ins, ScopedClock({None: tick_clock.global_clock})
        )
        si = drain_inst.ins.sync_info
        waits = list(si.on_wait) if si is not None else []
        MAXW = 1
        if len(waits) > MAXW:
            drain_inst.ins.sync_info = mybir.SyncInfo(
                on_wait=waits[:MAXW], on_update=list(si.on_update)
            )
            for i in range(MAXW, len(waits), MAXW):
                d2 = nc.sync.drain()
                d2.ins.sync_info = mybir.SyncInfo(
                    on_wait=waits[i : i + MAXW], on_update=[]
                )
        nc.all_engine_barrier()
        popped = nc._tile_sem_poison_stack.pop()
        assert popped is self._sem_poison
        nc.clear_and_free_semaphores(list(self.sems.allocated().values()))
        nc.all_engine_barrier()

    tile.TileContext._drain_and_barrier = _drain_and_barrier
    tile.TileContext._drain_patched = True


def _split_excess_waits(nc, max_waits=1):
    """walrus in this container has a tight per-instruction sync-wait slot
    limit; move excess waits onto same-engine nops preceding the instruction
    (same-engine queue order makes sequential waiting equivalent)."""
    for f in nc.m.functions:
        for bb in f.blocks:
            out = []
            changed = False
            for inst in bb.instructions:
                si = inst.sync_info
                waits = list(si.on_wait) if si is not None else []
                if len(waits) > max_waits:
                    lead = waits[: len(waits) - max_waits]
                    keep = waits[len(waits) - max_waits :]
                    for i in range(0, len(lead), max_waits):
                        nop = mybir.InstNoOp(
                            name=f"{inst.name}_w{i}", engine=inst.engine, ins=[], outs=[]
                        )
                        nop.sync_info = mybir.SyncInfo(
                            on_wait=lead[i : i + max_waits], on_update=[]
                        )
                        out.append(nop)
                    inst.sync_info = mybir.SyncInfo(
                        on_wait=keep, on_update=list(si.on_update)
                    )
                    changed = True
                out.append(inst)
            if changed:
                bb.instructions = out


def build_program(split_waits=True):
    _patch_tile_drain()
    nc = bass.Bass("TRN2", target_bir_lowering=False, debug=False, num_devices=N_CORES)

    xkv_d = nc.dram_tensor("xkv", [L, D], F32, kind="ExternalInput")
    wfc_d = nc.dram_tensor("wfc", [D, 3 * H * HD], F32, kind="ExternalInput")
    bfc_d = nc.dram_tensor("bfc", [3 * H * HD], F32, kind="ExternalInput")
    wfc2_d = nc.dram_tensor("wfc2", [H * HD, D], F32, kind="ExternalInput")
    bfc2_d = nc.dram_tensor("bfc2", [D], F32, kind="ExternalInput")
    ident_d = nc.dram_tensor("ident", [128, 128], F32, kind="ExternalInput")
    out_d = nc.dram_tensor("out", [WQ, D], F32, kind="ExternalOutput")

    NT = L // 128            # 16 token tiles
    ND = D // 128            # 8 d tiles
    NW = WQ // 128           # 8 query-token tiles
    NM = L // 128            # 16 key tiles

    with tile.TileContext(nc) as tc:
        pers = tc.alloc_tile_pool(name="pers", bufs=1)
        pmm = tc.alloc_tile_pool(name="pmm", bufs=2, space="PSUM")
        pu = tc.alloc_tile_pool(name="pu", bufs=2, space="PSUM")

        # --- constants ---
        ident = pers.tile([128, 128], BF16, tag="ident")
        nc.gpsimd.dma_start(ident[:, :], ident_d[:, :])
        ones = pers.tile([128, 128], BF16, tag="ones")
        nc.gpsimd.memset(ones[:, :], 1.0)
        eps = pers.tile([128, 1], F32, tag="eps")
        nc.gpsimd.memset(eps[:, :], LN_EPS)

        qkv_pool = tc.alloc_tile_pool(name="qkv", bufs=1)
        # q is stored zero-padded per head ([128,WQ] with only this head's 64
        # rows nonzero) so the scores matmul can use the full-K=128 kT pair as
        # stationary: K=64 stationaries with fresh weights cost ~2x (weight
        # load does not overlap the running matmul).
        qZ = [qkv_pool.tile([128, WQ], BF16, tag=f"qZ{h}", name=f"qZ{h}") for h in range(H)]
        kT = [qkv_pool.tile([128, L], BF16, tag=f"kT{i}", name=f"kT{i}") for i in range(ND)]
        vaug = [qkv_pool.tile([128, H * 65], BF16, tag=f"va{i}", name=f"va{i}") for i in range(NM)]
        # nv stored as head-pair tiles so out-proj accumulates with K=128
        nvP = [pers.tile([128, WQ], BF16, tag=f"nvp{e}", name=f"nvp{e}") for e in range(ND)]
        w2 = [pers.tile([128, D], BF16, tag=f"w2_{e}", name=f"w2_{e}") for e in range(ND)]
        b2 = pers.tile([1, D], BF16, tag="b2")

        # ---- phases 0-2 interleaved: transpose, v-proj, then per head-pair
        # q/k projection immediately followed by that pair's attention, so the
        # PE keeps dense work while ACT chews through the exps.
        with tc.tile_pool(name="ph12", bufs=1) as ph1:
            ph2 = ph1
            xkvT = [ph1.tile([128, L], BF16, tag=f"xkvT{i}", name=f"xkvT{i}") for i in range(ND)]

            wfc_r = wfc_d.rearrange("d (h c) -> d h c", c=3 * HD)
            bfc_r = bfc_d.rearrange("(h c) -> h c", c=3 * HD)

            wvs = {}
            for c2 in range(2):
                for kd in range(ND):
                    w = ph1.tile([128, 512], BF16, tag=f"wv{c2}_{kd}", name=f"wv{c2}_{kd}")
                    nc.gpsimd.dma_start(
                        w[:, :],
                        wfc_r[kd * 128 : (kd + 1) * 128, c2 * 8 : (c2 + 1) * 8, 2 * HD : 3 * HD],
                    )
                    wvs[(c2, kd)] = w
            bv = ph1.tile([1, H * HD], BF16, tag="bv")
            nc.gpsimd.dma_start(bv[:, :], bfc_r[:, 2 * HD : 3 * HD])

            # per token-tile: transpose x, then immediately project v for that
            # key-tile (keeps ACT fed from the very start)
            for ti in range(NT):
                xb = ph1.tile([128, D], BF16, tag="xb", bufs=2)
                nc.gpsimd.dma_start(xb[:, :], xkv_d[ti * 128 : (ti + 1) * 128, :])
                for kd in range(ND):
                    pt = pmm.tile([128, 128], BF16, tag="mm")
                    nc.tensor.transpose(
                        pt[:, :], xb[:, kd * 128 : (kd + 1) * 128], ident[:, :]
                    )
                    dst_sl = xkvT[kd][:, ti * 128 : (ti + 1) * 128]
                    if kd % 2 == 0:
                        nc.vector.tensor_copy(dst_sl, pt[:, :])
                    else:
                        nc.scalar.copy(dst_sl, pt[:, :])
                ps = pmm.tile([128, 1024], F32, tag="mm", name=f"vps{ti}")
                for c2 in range(2):
                    sl = slice(c2 * 512, (c2 + 1) * 512)
                    for kd in range(ND):
                        nc.tensor.matmul(
                            ps[:, sl],
                            xkvT[kd][:, ti * 128 : (ti + 1) * 128],
                            wvs[(c2, kd)][:, :],
                            start=(kd == 0),
                            stop=False,
                        )
                    nc.tensor.matmul(
                        ps[:, sl],
                        ones[0:1, 0:128],
                        bv[0:1, sl],
                        start=False,
                        stop=True,
                    )
                va = vaug[ti]
                va_r = va[:, :].rearrange("p (h c) -> p h c", c=65)
                nc.gpsimd.memset(va_r[:, :, 64:65], 1.0)
                nc.scalar.activation(
                    va_r[:, :, 0:64],
                    ps[:, :],
                    AF.Silu,
                )

            def project_qk(et):
                """q (et<ND) or k (et>=ND) projection for e-tile et%ND."""
                is_q = et < ND
                qi = et % ND
                c0 = 0 if is_q else HD
                if is_q:
                    nc.gpsimd.memset(qZ[2 * qi][:, :], 0.0)
                    nc.gpsimd.memset(qZ[2 * qi + 1][:, :], 0.0)
                wts = []
                for kd in range(ND):
                    w = ph1.tile([128, 128], BF16, tag="wqk", bufs=8, name=f"wqk{et}_{kd}")
                    nc.gpsimd.dma_start(
                        w[:, :],
                        wfc_r[kd * 128 : (kd + 1) * 128, 2 * qi : 2 * qi + 2, c0 : c0 + HD],
                    )
                    wts.append(w)
                bt = ph1.tile([128, 1], F32, tag="bqk", bufs=3, name=f"bqk{et}")
                nc.gpsimd.dma_start(bt[:, :], bfc_r[2 * qi : 2 * qi + 2, c0 : c0 + HD])
                ncols = WQ if is_q else L
                for half in range(ncols // 1024):
                    ps = pmm.tile([128, 1024], F32, tag="mm", name=f"qk{et}_{half}")
                    for tc2 in range(2):
                        t0 = half * 1024 + tc2 * 512
                        for kd in range(ND):
                            nc.tensor.matmul(
                                ps[:, tc2 * 512 : (tc2 + 1) * 512],
                                wts[kd][:, :],
                                xkvT[kd][:, t0 : t0 + 512],
                                start=(kd == 0),
                                stop=(kd == ND - 1),
                            )
                    if is_q:
                        for pi in range(2):
                            pr = pi * 64
                            nc.scalar.activation(
                                qZ[2 * qi + pi][pr : pr + 64, half * 1024 : (half + 1) * 1024],
                                ps[pr : pr + 64, :],
                                AF.Silu,
                                bias=bt[pr : pr + 64, :],
                            )
                    else:
                        nc.scalar.activation(
                            kT[qi][:, half * 1024 : (half + 1) * 1024],
                            ps[:, :],
                            AF.Silu,
                            bias=bt[:, :],
                        )

            def attn_mms(h):
                et = h // 2
                pr = (h % 2) * 64
                u = pu.tile([128, 1024], F32, tag="u", name=f"u{h}")
                for mt in range(NM):
                    ps = pmm.tile([128, 1024], F32, tag="mm", name=f"sc{h}_{mt}")
                    for wc in range(2):
                        nc.tensor.matmul(
                            ps[:, wc * 512 : (wc + 1) * 512],
                            kT[et][:, mt * 128 : (mt + 1) * 128],
                            qZ[h][:, wc * 512 : (wc + 1) * 512],
                            start=True,
                            stop=True,
                        )
                    ex = ph2.tile([128, 1024], BF16, tag="exp", bufs=3, name=f"ex{h}_{mt}")
                    nc.scalar.activation(ex[:, :], ps[:, :], AF.Exp, scale=SCALE)
                    for wc in range(2):
                        sl = slice(wc * 512, (wc + 1) * 512)
                        nc.tensor.matmul(
                            u[0:65, sl],
                            vaug[mt][:, h * 65 : (h + 1) * 65],
                            ex[:, sl],
                            start=(mt == 0),
                            stop=(mt == NM - 1),
                        )
                return u

            def normalize(h, u):
                """Pipelined softmax-denominator normalization: issued one head
                late so the DVE reciprocal completes while the next head's
                matmuls keep the PE stream busy."""
                lnt = ph2.tile([128, 512], F32, tag="lnt", bufs=2, name=f"lnt{h}")
                rc = ph2.tile([128, 512], BF16, tag="recip", bufs=2, name=f"rc{h}")
                bcs = ph2.tile([64, 1024], BF16, tag="bcs", bufs=2, name=f"bcs{h}")
                bc = pmm.tile([128, 1024], F32, tag="mm", name=f"bc{h}")
                # 1/S = exp(-ln S) on ACT: shares the exp table (no DVE
                # reciprocal, which costs ~6.5us and blocks the DVE queue)
                for wc in range(2):
                    sl = slice(wc * 512, (wc + 1) * 512)
                    lsl = slice(0, 512)
                    nc.scalar.activation(lnt[64:65, lsl], u[64:65, sl], AF.Ln)
                    nc.scalar.activation(rc[64:65, lsl], lnt[64:65, lsl], AF.Exp, scale=-1.0)
                    nc.tensor.matmul(
                        bc[0:64, sl],
                        ones[64:65, 0:64],
                        rc[64:65, lsl],
                        start=True,
                        stop=True,
                    )
                nc.vector.tensor_copy(bcs[0:64, :], bc[0:64, :])
                if h % 2 == 0:
                    nc.vector.tensor_mul(nvP[h // 2][0:64, :], u[0:64, :], bcs[0:64, :])
                else:
                    nvt = ph2.tile([64, 1024], BF16, tag="nvt", bufs=2, name=f"nvt{h}")
                    nc.vector.tensor_mul(nvt[:, :], u[0:64, :], bcs[0:64, :])
                    nc.vector.stream_shuffle(nvP[h // 2][64:128, :], nvt[0:64, :], list(range(32)))

            # prefetch output-projection weights well before use
            for e in range(ND):
                nc.gpsimd.dma_start(w2[e][:, :], wfc2_d[e * 128 : (e + 1) * 128, :])
            nc.gpsimd.dma_start(b2[:, :], bfc2_d[:])

            project_qk(0)
            project_qk(ND)
            pending = None
            for et in range(ND):
                if et + 1 < ND:
                    project_qk(et + 1)
                    project_qk(ND + et + 1)
                u_a = attn_mms(2 * et)
                if pending is not None:
                    normalize(*pending)
                u_b = attn_mms(2 * et + 1)
                normalize(2 * et, u_a)
                pending = (2 * et + 1, u_b)
            normalize(*pending)

        # ---------------- phase 3: output projection + LN ------------------
        with tc.tile_pool(name="ph3", bufs=1) as ph3:
            for wt in range(NW):
                po = pmm.tile([128, 1024], F32, tag="mm")
                for dc in range(2):
                    sl = slice(dc * 512, (dc + 1) * 512)
                    for e in range(ND):
                        nc.tensor.matmul(
                            po[:, sl],
                            nvP[e][:, wt * 128 : (wt + 1) * 128],
                            w2[e][:, sl],
                            start=(e == 0),
                            stop=False,
                        )
                    nc.tensor.matmul(
                        po[:, sl],
                        ones[0:1, 0:128],
                        b2[0:1, sl],
                        start=False,
                        stop=True,
                    )
                msb = ph3.tile([128, 1024], F32, tag="m", bufs=2)
                nc.scalar.activation(msb[:, :], po[:, :], AF.Silu)
                xr = ph3.tile([128, 1024], F32, tag="xr", bufs=2)
                nc.gpsimd.dma_start(xr[:, :], xkv_d[wt * 128 : (wt + 1) * 128, :])
                y = ph3.tile([128, 1024], F32, tag="y", bufs=2)
                nc.vector.tensor_add(y[:, :], msb[:, :], xr[:, :])
                st = ph3.tile([128, 12], F32, tag="st", bufs=2)
                nc.vector.bn_stats(st[:, 0:6], y[:, 0:512])
                nc.vector.bn_stats(st[:, 6:12], y[:, 512:1024])
                mv = ph3.tile([128, 2], F32, tag="mv", bufs=2)
                nc.vector.bn_aggr(mv[:, :], st[:, :])
                sd = ph3.tile([128, 2], F32, tag="sd", bufs=2)
                nc.scalar.activation(sd[:, 0:1], mv[:, 1:2], AF.Sqrt, bias=eps[:, 0:1])
                nc.vector.reciprocal(sd[:, 1:2], sd[:, 0:1])
                ot = ph3.tile([128, 1024], F32, tag="ot", bufs=2)
                nc.vector.tensor_scalar(
                    ot[:, :],
                    y[:, :],
                    mv[:, 0:1],
                    sd[:, 1:2],
                    ALU.subtract,
                    ALU.mult,
                )
                nc.gpsimd.dma_start(out_d[wt * 128 : (wt + 1) * 128, :], ot[:, :])

        qkv_pool.release()
        pu.release()
        pmm.release()
        pers.release()

    if split_waits:
        _split_excess_waits(nc)
    return nc


_NC_CACHE = None


def _get_program():
    global _NC_CACHE
    if _NC_CACHE is None:
        _NC_CACHE = build_program()
    return _NC_CACHE


def make_in_maps(x, W_fc, b_fc, W_fc2, b_fc2):
    x = np.asarray(x, dtype=np.float32)
    W_fc = np.ascontiguousarray(np.asarray(W_fc, dtype=np.float32))
    b_fc = np.ascontiguousarray(np.asarray(b_fc, dtype=np.float32))
    W_fc2 = np.ascontiguousarray(np.asarray(W_fc2, dtype=np.float32))
    b_fc2 = np.ascontiguousarray(np.asarray(b_fc2, dtype=np.float32))
    ident = np.eye(128, dtype=np.float32)
    in_maps = []
    for i in range(N_CORES):
        b = i // 2
        w0 = (i % 2) * WQ
        xkv = np.ascontiguousarray(
            np.concatenate([x[b, w0:], x[b, :w0]], axis=0)
        )
        in_maps.append(
            {
                "xkv": xkv,
                "wfc": W_fc,
                "bfc": b_fc,
                "wfc2": W_fc2,
                "bfc2": b_fc2,
                "ident": ident,
            }
        )
    return in_maps


def kernel(x, W_fc, b_fc, W_fc2, b_fc2, **extra):
    nc = _get_program()
    in_maps = make_in_maps(x, W_fc, b_fc, W_fc2, b_fc2)
    res = run_bass_kernel_spmd(nc, in_maps, list(range(N_CORES)))
    out = np.empty((B, L, D), dtype=np.float32)
    for i in range(N_CORES):
        b = i // 2
        w0 = (i % 2) * WQ
        out[b, w0 : w0 + WQ] = res.results[i]["out"]
    return out


# revision 18
# speedup vs baseline: 1.0381x; 1.0381x over previous
"""Trainium2 Bass kernel for nn_Attention_16612933500996.

Full-input contract: kernel(**inputs) takes the unsharded inputs and returns
the full output. Internally shards across 8 NeuronCores: core i handles
batch b = i//2 and query-half w = i%2 (1024 of 2048 tokens). No collectives:
each core recomputes K/V for its whole batch (x rows are rotated host-side so
each core's query tokens are always rows 0..1023 — softmax over keys is
permutation invariant).

Per-core pipeline (all matmuls bf16 -> f32 PSUM):
  0. PE-transpose x [t,d] -> xT [d,t] (bf16)
  1. QKV projection: qT/kT produced transposed ([head*64+c, t]); V produced
     natural ([t, head-major cols]) with a fused ones-column per head so the
     attention U-matmul also yields the softmax denominator row.
  2. Attention per head: scoresT[m,w] = kT.T @ qT; exp via ACT (scores are
     ~±0.8 so no max-subtraction needed); U[65,w] = v_aug.T @ exp accumulated
     over key tiles (row 64 = sum of exps); normalize U/S with a PE-broadcast
     reciprocal; result nvT[e,w].
  3. Output projection (per-head K=64 accumulation) + bias + swish + residual
     + layernorm, DMA out.
"""

import sys

sys.path.insert(0, "/opt/trn_rl_repo")

import numpy as np

import concourse.bass as bass
import concourse.tile as tile
from concourse import mybir
from concourse.bass_utils import run_bass_kernel_spmd

AF = mybir.ActivationFunctionType
ALU = mybir.AluOpType
F32 = mybir.dt.float32
F32R = mybir.dt.float32r
BF16 = mybir.dt.bfloat16

B, L, D = 4, 2048, 1024
H, HD = 16, 64
WQ = 1024          # query tokens per core
N_CORES = 8
SCALE = 1.0 / float(np.sqrt(np.float32(L)))
LN_EPS = 1e-5


def _patch_tile_drain():
    """walrus in this container only accepts 1 sem wait on the TPB_CTRL drain;
    split the TileContext tail-drain waits across multiple drain instructions."""
    if getattr(tile.TileContext, "_drain_patched", False):
        return
    from concourse.tile import ScopedClock

    def _drain_and_barrier(self, tick_clock, wait_clock):
        nc = self.nc
        drain_inst = nc.sync.drain()
        wait_clock.add_sem_waits(
            drain_inst.ins, ScopedClock({None: tick_clock.global_clock})
        )
        si = drain_inst.ins.sync_info
        waits = list(si.on_wait) if si is not None else []
        MAXW = 1
        if len(waits) > MAXW:
            drain_inst.ins.sync_info = mybir.SyncInfo(
                on_wait=waits[:MAXW], on_update=list(si.on_update)
            )
            for i in range(MAXW, len(waits), MAXW):
                d2 = nc.sync.drain()
                d2.ins.sync_info = mybir.SyncInfo(
                    on_wait=waits[i : i + MAXW], on_update=[]
                )
        nc.all_engine_barrier()
        popped = nc._tile_sem_poison_stack.pop()
        assert popped is self._sem_poison
        nc.clear_and_free_semaphores(list(self.sems.allocated().values()))
        nc.all_engine_barrier()

    tile.TileContext._drain_and_barrier = _drain_and_barrier
    tile.TileContext._drain_patched = True


def _split_excess_waits(nc, max_waits=1):
    """walrus in this container has a tight per-instruction sync-wait slot
    limit; move excess waits onto same-engine nops preceding the instruction
    (same-engine queue order makes sequential waiting equivalent)."""
    for f in nc.m.functions:
        for bb in f.blocks:
            out = []
            changed = False
            for inst in bb.instructions:
                si = inst.sync_info
                waits = list(si.on_wait) if si is not None else []
                if len(waits) > max_waits:
                    lead = waits[: len(waits) - max_waits]
                    keep = waits[len(waits) - max_waits :]
                    for i in range(0, len(lead), max_waits):
                        nop = mybir.InstNoOp(
                            name=f"{inst.name}_w{i}", engine=inst.engine, ins=[], outs=[]
                        )
                        nop.sync_info = mybir.SyncInfo(
                            on_wait=lead[i : i + max_waits], on_update=[]
                        )
                        out.append(nop)
                    inst.sync_info = mybir.SyncInfo(
                        on_wait=keep, on_update=list(si.on_update)
                    )
                    changed = True
                out.append(inst)
            if changed:
                bb.instructions = out


def build_program(split_waits=True):
    _patch_tile_drain()
    nc = bass.Bass("TRN2", target_bir_lowering=False, debug=False, num_devices=N_CORES)

    xkv_d = nc.dram_tensor("xkv", [L, D], F32, kind="ExternalInput")
    wfc_d = nc.dram_tensor("wfc", [D, 3 * H * HD], F32, kind="ExternalInput")
    bfc_d = nc.dram_tensor("bfc", [3 * H * HD], F32, kind="ExternalInput")
    wfc2_d = nc.dram_tensor("wfc2", [H * HD, D], F32, kind="ExternalInput")
    bfc2_d = nc.dram_tensor("bfc2", [D], F32, kind="ExternalInput")
    ident_d = nc.dram_tensor("ident", [128, 128], F32, kind="ExternalInput")
    out_d = nc.dram_tensor("out", [WQ, D], F32, kind="ExternalOutput")

    NT = L // 128            # 16 token tiles
    ND = D // 128            # 8 d tiles
    NW = WQ // 128           # 8 query-token tiles
    NM = L // 128            # 16 key tiles

    with tile.TileContext(nc) as tc:
        pers = tc.alloc_tile_pool(name="pers", bufs=1)
        pmm = tc.alloc_tile_pool(name="pmm", bufs=2, space="PSUM")
        pu = tc.alloc_tile_pool(name="pu", bufs=2, space="PSUM")

        # --- constants ---
        ident = pers.tile([128, 128], BF16, tag="ident")
        nc.gpsimd.dma_start(ident[:, :], ident_d[:, :])
        ones = pers.tile([128, 128], BF16, tag="ones")
        nc.gpsimd.memset(ones[:, :], 1.0)
        eps = pers.tile([128, 1], F32, tag="eps")
        nc.gpsimd.memset(eps[:, :], LN_EPS)

        qkv_pool = tc.alloc_tile_pool(name="qkv", bufs=1)
        # q is stored zero-padded per head ([128,WQ] with only this head's 64
        # rows nonzero) so the scores matmul can use the full-K=128 kT pair as
        # stationary: K=64 stationaries with fresh weights cost ~2x (weight
        # load does not overlap the running matmul).
        qZ = [qkv_pool.tile([128, WQ], BF16, tag=f"qZ{h}", name=f"qZ{h}") for h in range(H)]
        kT = [qkv_pool.tile([128, L], BF16, tag=f"kT{i}", name=f"kT{i}") for i in range(ND)]
        vaug = [qkv_pool.tile([128, H * 65], BF16, tag=f"va{i}", name=f"va{i}") for i in range(NM)]
        # nv stored as head-pair tiles so out-proj accumulates with K=128
        nvP = [pers.tile([128, WQ], BF16, tag=f"nvp{e}", name=f"nvp{e}") for e in range(ND)]
        w2 = [pers.tile([128, D], BF16, tag=f"w2_{e}", name=f"w2_{e}") for e in range(ND)]
        b2 = pers.tile([1, D], BF16, tag="b2")

        # ---- phases 0-2 interleaved: transpose, v-proj, then per head-pair
        # q/k projection immediately followed by that pair's attention, so the
        # PE keeps dense work while ACT chews through the exps.
        with tc.tile_pool(name="ph12", bufs=1) as ph1:
            ph2 = ph1
            xkvT = [ph1.tile([128, L], BF16, tag=f"xkvT{i}", name=f"xkvT{i}") for i in range(ND)]

            wfc_r = wfc_d.rearrange("d (h c) -> d h c", c=3 * HD)
            bfc_r = bfc_d.rearrange("(h c) -> h c", c=3 * HD)

            # first x tiles in flight before the wv loads queue behind them
            xb_pre = []
            for ti in range(2):
                xb = ph1.tile([128, D], BF16, tag="xb", bufs=2, name=f"xbp{ti}")
                nc.gpsimd.dma_start(xb[:, :], xkv_d[ti * 128 : (ti + 1) * 128, :])
                xb_pre.append(xb)
            wvs = {}
            for c2 in range(2):
                for kd in range(ND):
                    w = ph1.tile([128, 512], BF16, tag=f"wv{c2}_{kd}", name=f"wv{c2}_{kd}")
                    nc.gpsimd.dma_start(
                        w[:, :],
                        wfc_r[kd * 128 : (kd + 1) * 128, c2 * 8 : (c2 + 1) * 8, 2 * HD : 3 * HD],
                    )
                    wvs[(c2, kd)] = w
            bv = ph1.tile([1, H * HD], BF16, tag="bv")
            nc.gpsimd.dma_start(bv[:, :], bfc_r[:, 2 * HD : 3 * HD])

            # per token-tile: transpose x, then immediately project v for that
            # key-tile (keeps ACT fed from the very start)
            for ti in range(NT):
                if ti < 2:
                    xb = xb_pre[ti]
                else:
                    xb = ph1.tile([128, D], BF16, tag="xb", bufs=2)
                    nc.gpsimd.dma_start(xb[:, :], xkv_d[ti * 128 : (ti + 1) * 128, :])
                for kd in range(ND):
                    pt = pmm.tile([128, 128], BF16, tag="mm")
                    nc.tensor.transpose(
                        pt[:, :], xb[:, kd * 128 : (kd + 1) * 128], ident[:, :]
                    )
                    nc.vector.tensor_copy(
                        xkvT[kd][:, ti * 128 : (ti + 1) * 128], pt[:, :]
                    )
                ps = pmm.tile([128, 1024], F32, tag="mm", name=f"vps{ti}")
                for c2 in range(2):
                    sl = slice(c2 * 512, (c2 + 1) * 512)
                    for kd in range(ND):
                        nc.tensor.matmul(
                            ps[:, sl],
                            xkvT[kd][:, ti * 128 : (ti + 1) * 128],
                            wvs[(c2, kd)][:, :],
                            start=(kd == 0),
                            stop=False,
                        )
                    nc.tensor.matmul(
                        ps[:, sl],
                        ones[0:1, 0:128],
                        bv[0:1, sl],
                        start=False,
                        stop=True,
                    )
                va = vaug[ti]
                va_r = va[:, :].rearrange("p (h c) -> p h c", c=65)
                nc.gpsimd.memset(va_r[:, :, 64:65], 1.0)
                nc.scalar.activation(
                    va_r[:, :, 0:64],
                    ps[:, :],
                    AF.Silu,
                )

            def project_qk(et):
                """q (et<ND) or k (et>=ND) projection for e-tile et%ND."""
                is_q = et < ND
                qi = et % ND
                c0 = 0 if is_q else HD
                if is_q:
                    nc.gpsimd.memset(qZ[2 * qi][:, :], 0.0)
                    nc.gpsimd.memset(qZ[2 * qi + 1][:, :], 0.0)
                wts = []
                for kd in range(ND):
                    w = ph1.tile([128, 128], BF16, tag="wqk", bufs=8, name=f"wqk{et}_{kd}")
                    nc.gpsimd.dma_start(
                        w[:, :],
                        wfc_r[kd * 128 : (kd + 1) * 128, 2 * qi : 2 * qi + 2, c0 : c0 + HD],
                    )
                    wts.append(w)
                bt = ph1.tile([128, 1], F32, tag="bqk", bufs=3, name=f"bqk{et}")
                nc.gpsimd.dma_start(bt[:, :], bfc_r[2 * qi : 2 * qi + 2, c0 : c0 + HD])
                ncols = WQ if is_q else L
                for half in range(ncols // 1024):
                    ps = pmm.tile([128, 1024], F32, tag="mm", name=f"qk{et}_{half}")
                    for tc2 in range(2):
                        t0 = half * 1024 + tc2 * 512
                        for kd in range(ND):
                            nc.tensor.matmul(
                                ps[:, tc2 * 512 : (tc2 + 1) * 512],
                                wts[kd][:, :],
                                xkvT[kd][:, t0 : t0 + 512],
                                start=(kd == 0),
                                stop=(kd == ND - 1),
                            )
                    if is_q:
                        for pi in range(2):
                            pr = pi * 64
                            nc.scalar.activation(
                                qZ[2 * qi + pi][pr : pr + 64, half * 1024 : (half + 1) * 1024],
                                ps[pr : pr + 64, :],
                                AF.Silu,
                                bias=bt[pr : pr + 64, :],
                            )
                    else:
                        nc.scalar.activation(
                            kT[qi][:, half * 1024 : (half + 1) * 1024],
                            ps[:, :],
                            AF.Silu,
                            bias=bt[:, :],
                        )

            def attn_mms(h):
                et = h // 2
                pr = (h % 2) * 64
                u = pu.tile([128, 1024], F32, tag="u", name=f"u{h}")
                for mt in range(NM):
                    ps = pmm.tile([128, 1024], F32, tag="mm", name=f"sc{h}_{mt}")
                    for wc in range(2):
                        nc.tensor.matmul(
                            ps[:, wc * 512 : (wc + 1) * 512],
                            kT[et][:, mt * 128 : (mt + 1) * 128],
                            qZ[h][:, wc * 512 : (wc + 1) * 512],
                            start=True,
                            stop=True,
                        )
                    ex = ph2.tile([128, 1024], BF16, tag="exp", bufs=3, name=f"ex{h}_{mt}")
                    nc.scalar.activation(ex[:, :], ps[:, :], AF.Exp, scale=SCALE)
                    for wc in range(2):
                        sl = slice(wc * 512, (wc + 1) * 512)
                        nc.tensor.matmul(
                            u[0:65, sl],
                            vaug[mt][:, h * 65 : (h + 1) * 65],
                            ex[:, sl],
                            start=(mt == 0),
                            stop=(mt == NM - 1),
                        )
                return u

            def normalize(h, u):
                """Pipelined softmax-denominator normalization: issued one head
                late so the DVE reciprocal completes while the next head's
                matmuls keep the PE stream busy."""
                lnt = ph2.tile([128, 512], F32, tag="lnt", bufs=2, name=f"lnt{h}")
                rc = ph2.tile([128, 512], BF16, tag="recip", bufs=2, name=f"rc{h}")
                bcs = ph2.tile([64, 1024], BF16, tag="bcs", bufs=2, name=f"bcs{h}")
                bc = pmm.tile([128, 1024], F32, tag="mm", name=f"bc{h}")
                # 1/S = exp(-ln S) on ACT: shares the exp table (no DVE
                # reciprocal, which costs ~6.5us and blocks the DVE queue)
                for wc in range(2):
                    sl = slice(wc * 512, (wc + 1) * 512)
                    lsl = slice(0, 512)
                    nc.scalar.activation(lnt[64:65, lsl], u[64:65, sl], AF.Ln)
                    nc.scalar.activation(rc[64:65, lsl], lnt[64:65, lsl], AF.Exp, scale=-1.0)
                    nc.tensor.matmul(
                        bc[0:64, sl],
                        ones[64:65, 0:64],
                        rc[64:65, lsl],
                        start=True,
                        stop=True,
                    )
                nc.vector.tensor_copy(bcs[0:64, :], bc[0:64, :])
                if h % 2 == 0:
                    nc.vector.tensor_mul(nvP[h // 2][0:64, :], u[0:64, :], bcs[0:64, :])
                else:
                    nvt = ph2.tile([64, 1024], BF16, tag="nvt", bufs=2, name=f"nvt{h}")
                    nc.vector.tensor_mul(nvt[:, :], u[0:64, :], bcs[0:64, :])
                    nc.vector.stream_shuffle(nvP[h // 2][64:128, :], nvt[0:64, :], list(range(32)))

            # prefetch output-projection weights well before use
            for e in range(ND):
                nc.gpsimd.dma_start(w2[e][:, :], wfc2_d[e * 128 : (e + 1) * 128, :])
            nc.gpsimd.dma_start(b2[:, :], bfc2_d[:])

            project_qk(0)
            project_qk(ND)
            pending = None
            for et in range(ND):
                if et + 1 < ND:
                    project_qk(et + 1)
                    project_qk(ND + et + 1)
                u_a = attn_mms(2 * et)
                if pending is not None:
                    normalize(*pending)
                u_b = attn_mms(2 * et + 1)
                normalize(2 * et, u_a)
                pending = (2 * et + 1, u_b)
            normalize(*pending)

        # ---------------- phase 3: output projection + LN ------------------
        with tc.tile_pool(name="ph3", bufs=1) as ph3:
            for wt in range(NW):
                po = pmm.tile([128, 1024], F32, tag="mm")
                for dc in range(2):
                    sl = slice(dc * 512, (dc + 1) * 512)
                    for e in range(ND):
                        nc.tensor.matmul(
                            po[:, sl],
                            nvP[e][:, wt * 128 : (wt + 1) * 128],
                            w2[e][:, sl],
                            start=(e == 0),
                            stop=False,
                        )
                    nc.tensor.matmul(
                        po[:, sl],
                        ones[0:1, 0:128],
                        b2[0:1, sl],
                        start=False,
                        stop=True,
                    )
                msb = ph3.tile([128, 1024], F32, tag="m", bufs=2)
                nc.scalar.activation(msb[:, :], po[:, :], AF.Silu)
                xr = ph3.tile([128, 1024], F32, tag="xr", bufs=2)
                nc.gpsimd.dma_start(xr[:, :], xkv_d[wt * 128 : (wt + 1) * 128, :])
                y = ph3.tile([128, 1024], F32, tag="y", bufs=2)
                nc.vector.tensor_add(y[:, :], msb[:, :], xr[:, :])
                st = ph3.tile([128, 12], F32, tag="st", bufs=2)
                nc.vector.bn_stats(st[:, 0:6], y[:, 0:512])
                nc.vector.bn_stats(st[:, 6:12], y[:, 512:1024])
                mv = ph3.tile([128, 2], F32, tag="mv", bufs=2)
                nc.vector.bn_aggr(mv[:, :], st[:, :])
                sd = ph3.tile([128, 2], F32, tag="sd", bufs=2)
                nc.scalar.activation(sd[:, 0:1], mv[:, 1:2], AF.Sqrt, bias=eps[:, 0:1])
                nc.vector.reciprocal(sd[:, 1:2], sd[:, 0:1])
                ot = ph3.tile([128, 1024], F32, tag="ot", bufs=2)
                nc.vector.tensor_scalar(
                    ot[:, :],
                    y[:, :],
                    mv[:, 0:1],
                    sd[:, 1:2],
                    ALU.subtract,
                    ALU.mult,
                )
                nc.gpsimd.dma_start(out_d[wt * 128 : (wt + 1) * 128, :], ot[:, :])

        qkv_pool.release()
        pu.release()
        pmm.release()
        pers.release()

    if split_waits:
        _split_excess_waits(nc)
    return nc


_NC_CACHE = None


def _get_program():
    global _NC_CACHE
    if _NC_CACHE is None:
        _NC_CACHE = build_program()
    return _NC_CACHE


def make_in_maps(x, W_fc, b_fc, W_fc2, b_fc2):
    x = np.asarray(x, dtype=np.float32)
    W_fc = np.ascontiguousarray(np.asarray(W_fc, dtype=np.float32))
    b_fc = np.ascontiguousarray(np.asarray(b_fc, dtype=np.float32))
    W_fc2 = np.ascontiguousarray(np.asarray(W_fc2, dtype=np.float32))
    b_fc2 = np.ascontiguousarray(np.asarray(b_fc2, dtype=np.float32))
    ident = np.eye(128, dtype=np.float32)
    in_maps = []
    for i in range(N_CORES):
        b = i // 2
        w0 = (i % 2) * WQ
        xkv = np.ascontiguousarray(
            np.concatenate([x[b, w0:], x[b, :w0]], axis=0)
        )
        in_maps.append(
            {
                "xkv": xkv,
                "wfc": W_fc,
                "bfc": b_fc,
                "wfc2": W_fc2,
                "bfc2": b_fc2,
                "ident": ident,
            }
        )
    return in_maps


def kernel(x, W_fc, b_fc, W_fc2, b_fc2, **extra):
    nc = _get_program()
    in_maps = make_in_maps(x, W_fc, b_fc, W_fc2, b_fc2)
    res = run_bass_kernel_spmd(nc, in_maps, list(range(N_CORES)))
    out = np.empty((B, L, D), dtype=np.float32)
    for i in range(N_CORES):
        b = i // 2
        w0 = (i % 2) * WQ
        out[b, w0 : w0 + WQ] = res.results[i]["out"]
    return out


# revision 19
# speedup vs baseline: 1.0416x; 1.0034x over previous
"""Trainium2 Bass kernel for nn_Attention_16612933500996.

Full-input contract: kernel(**inputs) takes the unsharded inputs and returns
the full output. Internally shards across 8 NeuronCores: core i handles
batch b = i//2 and query-half w = i%2 (1024 of 2048 tokens). No collectives:
each core recomputes K/V for its whole batch (x rows are rotated host-side so
each core's query tokens are always rows 0..1023 — softmax over keys is
permutation invariant).

Per-core pipeline (all matmuls bf16 -> f32 PSUM):
  0. PE-transpose x [t,d] -> xT [d,t] (bf16)
  1. QKV projection: qT/kT produced transposed ([head*64+c, t]); V produced
     natural ([t, head-major cols]) with a fused ones-column per head so the
     attention U-matmul also yields the softmax denominator row.
  2. Attention per head: scoresT[m,w] = kT.T @ qT; exp via ACT (scores are
     ~±0.8 so no max-subtraction needed); U[65,w] = v_aug.T @ exp accumulated
     over key tiles (row 64 = sum of exps); normalize U/S with a PE-broadcast
     reciprocal; result nvT[e,w].
  3. Output projection (per-head K=64 accumulation) + bias + swish + residual
     + layernorm, DMA out.
"""

import sys

sys.path.insert(0, "/opt/trn_rl_repo")

import numpy as np

import concourse.bass as bass
import concourse.tile as tile
from concourse import mybir
from concourse.bass_utils import run_bass_kernel_spmd

AF = mybir.ActivationFunctionType
ALU = mybir.AluOpType
F32 = mybir.dt.float32
F32R = mybir.dt.float32r
BF16 = mybir.dt.bfloat16

B, L, D = 4, 2048, 1024
H, HD = 16, 64
WQ = 1024          # query tokens per core
N_CORES = 8
SCALE = 1.0 / float(np.sqrt(np.float32(L)))
LN_EPS = 1e-5


def _patch_tile_drain():
    """walrus in this container only accepts 1 sem wait on the TPB_CTRL drain;
    split the TileContext tail-drain waits across multiple drain instructions."""
    if getattr(tile.TileContext, "_drain_patched", False):
        return
    from concourse.tile import ScopedClock

    def _drain_and_barrier(self, tick_clock, wait_clock):
        nc = self.nc
        drain_inst = nc.sync.drain()
        wait_clock.add_sem_waits(
            drain_inst.ins, ScopedClock({None: tick_clock.global_clock})
        )
        si = drain_inst.ins.sync_info
        waits = list(si.on_wait) if si is not None else []
        MAXW = 1
        if len(waits) > MAXW:
            drain_inst.ins.sync_info = mybir.SyncInfo(
                on_wait=waits[:MAXW], on_update=list(si.on_update)
            )
            for i in range(MAXW, len(waits), MAXW):
                d2 = nc.sync.drain()
                d2.ins.sync_info = mybir.SyncInfo(
                    on_wait=waits[i : i + MAXW], on_update=[]
                )
        nc.all_engine_barrier()
        popped = nc._tile_sem_poison_stack.pop()
        assert popped is self._sem_poison
        nc.clear_and_free_semaphores(list(self.sems.allocated().values()))
        nc.all_engine_barrier()

    tile.TileContext._drain_and_barrier = _drain_and_barrier
    tile.TileContext._drain_patched = True


def _split_excess_waits(nc, max_waits=1):
    """walrus in this container has a tight per-instruction sync-wait slot
    limit; move excess waits onto same-engine nops preceding the instruction
    (same-engine queue order makes sequential waiting equivalent)."""
    for f in nc.m.functions:
        for bb in f.blocks:
            out = []
            changed = False
            for inst in bb.instructions:
                si = inst.sync_info
                waits = list(si.on_wait) if si is not None else []
                if len(waits) > max_waits:
                    lead = waits[: len(waits) - max_waits]
                    keep = waits[len(waits) - max_waits :]
                    for i in range(0, len(lead), max_waits):
                        nop = mybir.InstNoOp(
                            name=f"{inst.name}_w{i}", engine=inst.engine, ins=[], outs=[]
                        )
                        nop.sync_info = mybir.SyncInfo(
                            on_wait=lead[i : i + max_waits], on_update=[]
                        )
                        out.append(nop)
                    inst.sync_info = mybir.SyncInfo(
                        on_wait=keep, on_update=list(si.on_update)
                    )
                    changed = True
                out.append(inst)
            if changed:
                bb.instructions = out


def build_program(split_waits=True):
    _patch_tile_drain()
    nc = bass.Bass("TRN2", target_bir_lowering=False, debug=False, num_devices=N_CORES)

    xkv_d = nc.dram_tensor("xkv", [L, D], F32, kind="ExternalInput")
    wfc_d = nc.dram_tensor("wfc", [D, 3 * H * HD], F32, kind="ExternalInput")
    bfc_d = nc.dram_tensor("bfc", [3 * H * HD], F32, kind="ExternalInput")
    wfc2_d = nc.dram_tensor("wfc2", [H * HD, D], F32, kind="ExternalInput")
    bfc2_d = nc.dram_tensor("bfc2", [D], F32, kind="ExternalInput")
    ident_d = nc.dram_tensor("ident", [128, 128], F32, kind="ExternalInput")
    out_d = nc.dram_tensor("out", [WQ, D], F32, kind="ExternalOutput")

    NT = L // 128            # 16 token tiles
    ND = D // 128            # 8 d tiles
    NW = WQ // 128           # 8 query-token tiles
    NM = L // 128            # 16 key tiles

    with tile.TileContext(nc) as tc:
        pers = tc.alloc_tile_pool(name="pers", bufs=1)
        pmm = tc.alloc_tile_pool(name="pmm", bufs=2, space="PSUM")
        pu = tc.alloc_tile_pool(name="pu", bufs=2, space="PSUM")

        # --- constants ---
        ident = pers.tile([128, 128], BF16, tag="ident")
        nc.gpsimd.dma_start(ident[:, :], ident_d[:, :])
        ones = pers.tile([128, 128], BF16, tag="ones")
        nc.gpsimd.memset(ones[:, :], 1.0)
        eps = pers.tile([128, 1], F32, tag="eps")
        nc.gpsimd.memset(eps[:, :], LN_EPS)

        qkv_pool = tc.alloc_tile_pool(name="qkv", bufs=1)
        # q is stored zero-padded per head ([128,WQ] with only this head's 64
        # rows nonzero) so the scores matmul can use the full-K=128 kT pair as
        # stationary: K=64 stationaries with fresh weights cost ~2x (weight
        # load does not overlap the running matmul).
        qZ = [qkv_pool.tile([128, WQ], BF16, tag=f"qZ{h}", name=f"qZ{h}") for h in range(H)]
        kT = [qkv_pool.tile([128, L], BF16, tag=f"kT{i}", name=f"kT{i}") for i in range(ND)]
        vaug = [qkv_pool.tile([128, H * 65], BF16, tag=f"va{i}", name=f"va{i}") for i in range(NM)]
        # nv stored as head-pair tiles so out-proj accumulates with K=128
        nvP = [pers.tile([128, WQ], BF16, tag=f"nvp{e}", name=f"nvp{e}") for e in range(ND)]
        w2 = [pers.tile([128, D], BF16, tag=f"w2_{e}", name=f"w2_{e}") for e in range(ND)]
        b2 = pers.tile([1, D], BF16, tag="b2")

        # ---- phases 0-2 interleaved: transpose, v-proj, then per head-pair
        # q/k projection immediately followed by that pair's attention, so the
        # PE keeps dense work while ACT chews through the exps.
        with tc.tile_pool(name="ph12", bufs=1) as ph1:
            ph2 = ph1
            xkvT = [ph1.tile([128, L], BF16, tag=f"xkvT{i}", name=f"xkvT{i}") for i in range(ND)]

            wfc_r = wfc_d.rearrange("d (h c) -> d h c", c=3 * HD)
            bfc_r = bfc_d.rearrange("(h c) -> h c", c=3 * HD)

            # first x tiles in flight before the wv loads queue behind them
            xb_pre = []
            for ti in range(2):
                xb = ph1.tile([128, D], BF16, tag="xb", bufs=2, name=f"xbp{ti}")
                nc.gpsimd.dma_start(xb[:, :], xkv_d[ti * 128 : (ti + 1) * 128, :])
                xb_pre.append(xb)
            wvs = {}
            for c2 in range(2):
                for kd in range(ND):
                    w = ph1.tile([128, 512], BF16, tag=f"wv{c2}_{kd}", name=f"wv{c2}_{kd}")
                    nc.gpsimd.dma_start(
                        w[:, :],
                        wfc_r[kd * 128 : (kd + 1) * 128, c2 * 8 : (c2 + 1) * 8, 2 * HD : 3 * HD],
                    )
                    wvs[(c2, kd)] = w
            bv = ph1.tile([1, H * HD], BF16, tag="bv")
            nc.gpsimd.dma_start(bv[:, :], bfc_r[:, 2 * HD : 3 * HD])

            # per token-tile: transpose x, then immediately project v for that
            # key-tile (keeps ACT fed from the very start)
            for ti in range(NT):
                if ti < 2:
                    xb = xb_pre[ti]
                else:
                    xb = ph1.tile([128, D], BF16, tag="xb", bufs=2)
                    nc.gpsimd.dma_start(xb[:, :], xkv_d[ti * 128 : (ti + 1) * 128, :])
                for kd in range(ND):
                    pt = pmm.tile([128, 128], BF16, tag="mm")
                    nc.tensor.transpose(
                        pt[:, :], xb[:, kd * 128 : (kd + 1) * 128], ident[:, :]
                    )
                    nc.vector.tensor_copy(
                        xkvT[kd][:, ti * 128 : (ti + 1) * 128], pt[:, :]
                    )
                ps = pmm.tile([128, 1024], F32, tag="mm", name=f"vps{ti}")
                for c2 in range(2):
                    sl = slice(c2 * 512, (c2 + 1) * 512)
                    for kd in range(ND):
                        nc.tensor.matmul(
                            ps[:, sl],
                            xkvT[kd][:, ti * 128 : (ti + 1) * 128],
                            wvs[(c2, kd)][:, :],
                            start=(kd == 0),
                            stop=False,
                        )
                    nc.tensor.matmul(
                        ps[:, sl],
                        ones[0:1, 0:128],
                        bv[0:1, sl],
                        start=False,
                        stop=True,
                    )
                va = vaug[ti]
                va_r = va[:, :].rearrange("p (h c) -> p h c", c=65)
                nc.gpsimd.memset(va_r[:, :, 64:65], 1.0)
                nc.scalar.activation(
                    va_r[:, :, 0:64],
                    ps[:, :],
                    AF.Silu,
                )

            def project_qk(et):
                """q (et<ND) or k (et>=ND) projection for e-tile et%ND."""
                is_q = et < ND
                qi = et % ND
                c0 = 0 if is_q else HD
                if is_q:
                    nc.gpsimd.memset(qZ[2 * qi][:, :], 0.0)
                    nc.gpsimd.memset(qZ[2 * qi + 1][:, :], 0.0)
                wts = []
                for kd in range(ND):
                    w = ph1.tile([128, 128], BF16, tag="wqk", bufs=8, name=f"wqk{et}_{kd}")
                    nc.gpsimd.dma_start(
                        w[:, :],
                        wfc_r[kd * 128 : (kd + 1) * 128, 2 * qi : 2 * qi + 2, c0 : c0 + HD],
                    )
                    wts.append(w)
                bt = ph1.tile([128, 1], F32, tag="bqk", bufs=3, name=f"bqk{et}")
                nc.gpsimd.dma_start(bt[:, :], bfc_r[2 * qi : 2 * qi + 2, c0 : c0 + HD])
                ncols = WQ if is_q else L
                for half in range(ncols // 1024):
                    ps = pmm.tile([128, 1024], F32, tag="mm", name=f"qk{et}_{half}")
                    for tc2 in range(2):
                        t0 = half * 1024 + tc2 * 512
                        for kd in range(ND):
                            nc.tensor.matmul(
                                ps[:, tc2 * 512 : (tc2 + 1) * 512],
                                wts[kd][:, :],
                                xkvT[kd][:, t0 : t0 + 512],
                                start=(kd == 0),
                                stop=(kd == ND - 1),
                            )
                    if is_q:
                        for pi in range(2):
                            pr = pi * 64
                            nc.scalar.activation(
                                qZ[2 * qi + pi][pr : pr + 64, half * 1024 : (half + 1) * 1024],
                                ps[pr : pr + 64, :],
                                AF.Silu,
                                bias=bt[pr : pr + 64, :],
                            )
                    else:
                        nc.scalar.activation(
                            kT[qi][:, half * 1024 : (half + 1) * 1024],
                            ps[:, :],
                            AF.Silu,
                            bias=bt[:, :],
                        )

            def attn_mms(h):
                et = h // 2
                pr = (h % 2) * 64
                u = pu.tile([128, 1024], F32, tag="u", name=f"u{h}")
                # process key-tiles in pairs: both scores matmuls, both exps,
                # then both U matmuls — halves the stationary-shape transitions
                # on the PE (each scores->U switch costs ~160ns of weight-load)
                for mp in range(NM // 2):
                    exs = []
                    for mt in (2 * mp, 2 * mp + 1):
                        ps = pmm.tile([128, 1024], F32, tag="mm", name=f"sc{h}_{mt}")
                        for wc in range(2):
                            nc.tensor.matmul(
                                ps[:, wc * 512 : (wc + 1) * 512],
                                kT[et][:, mt * 128 : (mt + 1) * 128],
                                qZ[h][:, wc * 512 : (wc + 1) * 512],
                                start=True,
                                stop=True,
                            )
                        ex = ph2.tile([128, 1024], BF16, tag="exp", bufs=3, name=f"ex{h}_{mt}")
                        nc.scalar.activation(ex[:, :], ps[:, :], AF.Exp, scale=SCALE)
                        exs.append(ex)
                    for i, mt in enumerate((2 * mp, 2 * mp + 1)):
                        for wc in range(2):
                            sl = slice(wc * 512, (wc + 1) * 512)
                            nc.tensor.matmul(
                                u[0:65, sl],
                                vaug[mt][:, h * 65 : (h + 1) * 65],
                                exs[i][:, sl],
                                start=(mt == 0),
                                stop=(mt == NM - 1),
                            )
                return u

            def normalize(h, u):
                """Pipelined softmax-denominator normalization: issued one head
                late so the DVE reciprocal completes while the next head's
                matmuls keep the PE stream busy."""
                lnt = ph2.tile([128, 512], F32, tag="lnt", bufs=2, name=f"lnt{h}")
                rc = ph2.tile([128, 512], BF16, tag="recip", bufs=2, name=f"rc{h}")
                bcs = ph2.tile([64, 1024], BF16, tag="bcs", bufs=2, name=f"bcs{h}")
                bc = pmm.tile([128, 1024], F32, tag="mm", name=f"bc{h}")
                # 1/S = exp(-ln S) on ACT: shares the exp table (no DVE
                # reciprocal, which costs ~6.5us and blocks the DVE queue)
                for wc in range(2):
                    sl = slice(wc * 512, (wc + 1) * 512)
                    lsl = slice(0, 512)
                    nc.scalar.activation(lnt[64:65, lsl], u[64:65, sl], AF.Ln)
                    nc.scalar.activation(rc[64:65, lsl], lnt[64:65, lsl], AF.Exp, scale=-1.0)
                    nc.tensor.matmul(
                        bc[0:64, sl],
                        ones[64:65, 0:64],
                        rc[64:65, lsl],
                        start=True,
                        stop=True,
                    )
                nc.vector.tensor_copy(bcs[0:64, :], bc[0:64, :])
                if h % 2 == 0:
                    nc.vector.tensor_mul(nvP[h // 2][0:64, :], u[0:64, :], bcs[0:64, :])
                else:
                    nvt = ph2.tile([64, 1024], BF16, tag="nvt", bufs=2, name=f"nvt{h}")
                    nc.vector.tensor_mul(nvt[:, :], u[0:64, :], bcs[0:64, :])
                    nc.vector.stream_shuffle(nvP[h // 2][64:128, :], nvt[0:64, :], list(range(32)))

            # prefetch output-projection weights well before use
            for e in range(ND):
                nc.gpsimd.dma_start(w2[e][:, :], wfc2_d[e * 128 : (e + 1) * 128, :])
            nc.gpsimd.dma_start(b2[:, :], bfc2_d[:])

            project_qk(0)
            project_qk(ND)
            pending = None
            for et in range(ND):
                if et + 1 < ND:
                    project_qk(et + 1)
                    project_qk(ND + et + 1)
                u_a = attn_mms(2 * et)
                if pending is not None:
                    normalize(*pending)
                u_b = attn_mms(2 * et + 1)
                normalize(2 * et, u_a)
                pending = (2 * et + 1, u_b)
            normalize(*pending)

        # ---------------- phase 3: output projection + LN ------------------
        with tc.tile_pool(name="ph3", bufs=1) as ph3:
            for wt in range(NW):
                po = pmm.tile([128, 1024], F32, tag="mm")
                for dc in range(2):
                    sl = slice(dc * 512, (dc + 1) * 512)
                    for e in range(ND):
                        nc.tensor.matmul(
                            po[:, sl],
                            nvP[e][:, wt * 128 : (wt + 1) * 128],
                            w2[e][:, sl],
                            start=(e == 0),
                            stop=False,
                        )
                    nc.tensor.matmul(
                        po[:, sl],
                        ones[0:1, 0:128],
                        b2[0:1, sl],
                        start=False,
                        stop=True,
                    )
                msb = ph3.tile([128, 1024], F32, tag="m", bufs=2)
                nc.scalar.activation(msb[:, :], po[:, :], AF.Silu)
                xr = ph3.tile([128, 1024], F32, tag="xr", bufs=2)
                nc.gpsimd.dma_start(xr[:, :], xkv_d[wt * 128 : (wt + 1) * 128, :])
                y = ph3.tile([128, 1024], F32, tag="y", bufs=2)
                nc.vector.tensor_add(y[:, :], msb[:, :], xr[:, :])
                st = ph3.tile([128, 12], F32, tag="st", bufs=2)
                nc.vector.bn_stats(st[:, 0:6], y[:, 0:512])
                nc.vector.bn_stats(st[:, 6:12], y[:, 512:1024])
                mv = ph3.tile([128, 2], F32, tag="mv", bufs=2)
                nc.vector.bn_aggr(mv[:, :], st[:, :])
                sd = ph3.tile([128, 2], F32, tag="sd", bufs=2)
                nc.scalar.activation(sd[:, 0:1], mv[:, 1:2], AF.Sqrt, bias=eps[:, 0:1])
                nc.vector.reciprocal(sd[:, 1:2], sd[:, 0:1])
                ot = ph3.tile([128, 1024], F32, tag="ot", bufs=2)
                nc.vector.tensor_scalar(
                    ot[:, :],
                    y[:, :],
                    mv[:, 0:1],
                    sd[:, 1:2],
                    ALU.subtract,
                    ALU.mult,
                )
                nc.gpsimd.dma_start(out_d[wt * 128 : (wt + 1) * 128, :], ot[:, :])

        qkv_pool.release()
        pu.release()
        pmm.release()
        pers.release()

    if split_waits:
        _split_excess_waits(nc)
    return nc


_NC_CACHE = None


def _get_program():
    global _NC_CACHE
    if _NC_CACHE is None:
        _NC_CACHE = build_program()
    return _NC_CACHE


def make_in_maps(x, W_fc, b_fc, W_fc2, b_fc2):
    x = np.asarray(x, dtype=np.float32)
    W_fc = np.ascontiguousarray(np.asarray(W_fc, dtype=np.float32))
    b_fc = np.ascontiguousarray(np.asarray(b_fc, dtype=np.float32))
    W_fc2 = np.ascontiguousarray(np.asarray(W_fc2, dtype=np.float32))
    b_fc2 = np.ascontiguousarray(np.asarray(b_fc2, dtype=np.float32))
    ident = np.eye(128, dtype=np.float32)
    in_maps = []
    for i in range(N_CORES):
        b = i // 2
        w0 = (i % 2) * WQ
        xkv = np.ascontiguousarray(
            np.concatenate([x[b, w0:], x[b, :w0]], axis=0)
        )
        in_maps.append(
            {
                "xkv": xkv,
                "wfc": W_fc,
                "bfc": b_fc,
                "wfc2": W_fc2,
                "bfc2": b_fc2,
                "ident": ident,
            }
        )
    return in_maps


def kernel(x, W_fc, b_fc, W_fc2, b_fc2, **extra):
    nc = _get_program()
    in_maps = make_in_maps(x, W_fc, b_fc, W_fc2, b_fc2)
    res = run_bass_kernel_spmd(nc, in_maps, list(range(N_CORES)))
    out = np.empty((B, L, D), dtype=np.float32)
    for i in range(N_CORES):
        b = i // 2
        w0 = (i % 2) * WQ
        out[b, w0 : w0 + WQ] = res.results[i]["out"]
    return out


# revision 21
# speedup vs baseline: 1.0433x; 1.0016x over previous
"""Trainium2 Bass kernel for nn_Attention_16612933500996.

Full-input contract: kernel(**inputs) takes the unsharded inputs and returns
the full output. Internally shards across 8 NeuronCores: core i handles
batch b = i//2 and query-half w = i%2 (1024 of 2048 tokens). No collectives:
each core recomputes K/V for its whole batch (x rows are rotated host-side so
each core's query tokens are always rows 0..1023 — softmax over keys is
permutation invariant).

Per-core pipeline (all matmuls bf16 -> f32 PSUM):
  0. PE-transpose x [t,d] -> xT [d,t] (bf16)
  1. QKV projection: qT/kT produced transposed ([head*64+c, t]); V produced
     natural ([t, head-major cols]) with a fused ones-column per head so the
     attention U-matmul also yields the softmax denominator row.
  2. Attention per head: scoresT[m,w] = kT.T @ qT; exp via ACT (scores are
     ~±0.8 so no max-subtraction needed); U[65,w] = v_aug.T @ exp accumulated
     over key tiles (row 64 = sum of exps); normalize U/S with a PE-broadcast
     reciprocal; result nvT[e,w].
  3. Output projection (per-head K=64 accumulation) + bias + swish + residual
     + layernorm, DMA out.
"""

import sys

sys.path.insert(0, "/opt/trn_rl_repo")

import numpy as np

import concourse.bass as bass
import concourse.tile as tile
from concourse import mybir
from concourse.bass_utils import run_bass_kernel_spmd

AF = mybir.ActivationFunctionType
ALU = mybir.AluOpType
F32 = mybir.dt.float32
F32R = mybir.dt.float32r
BF16 = mybir.dt.bfloat16

B, L, D = 4, 2048, 1024
H, HD = 16, 64
WQ = 1024          # query tokens per core
N_CORES = 8
SCALE = 1.0 / float(np.sqrt(np.float32(L)))
LN_EPS = 1e-5


def _patch_tile_drain():
    """walrus in this container only accepts 1 sem wait on the TPB_CTRL drain;
    split the TileContext tail-drain waits across multiple drain instructions."""
    if getattr(tile.TileContext, "_drain_patched", False):
        return
    from concourse.tile import ScopedClock

    def _drain_and_barrier(self, tick_clock, wait_clock):
        nc = self.nc
        drain_inst = nc.sync.drain()
        wait_clock.add_sem_waits(
            drain_inst.ins, ScopedClock({None: tick_clock.global_clock})
        )
        si = drain_inst.ins.sync_info
        waits = list(si.on_wait) if si is not None else []
        MAXW = 1
        if len(waits) > MAXW:
            drain_inst.ins.sync_info = mybir.SyncInfo(
                on_wait=waits[:MAXW], on_update=list(si.on_update)
            )
            for i in range(MAXW, len(waits), MAXW):
                d2 = nc.sync.drain()
                d2.ins.sync_info = mybir.SyncInfo(
                    on_wait=waits[i : i + MAXW], on_update=[]
                )
        nc.all_engine_barrier()
        popped = nc._tile_sem_poison_stack.pop()
        assert popped is self._sem_poison
        nc.clear_and_free_semaphores(list(self.sems.allocated().values()))
        nc.all_engine_barrier()

    tile.TileContext._drain_and_barrier = _drain_and_barrier
    tile.TileContext._drain_patched = True


def _split_excess_waits(nc, max_waits=1):
    """walrus in this container has a tight per-instruction sync-wait slot
    limit; move excess waits onto same-engine nops preceding the instruction
    (same-engine queue order makes sequential waiting equivalent)."""
    for f in nc.m.functions:
        for bb in f.blocks:
            out = []
            changed = False
            for inst in bb.instructions:
                si = inst.sync_info
                waits = list(si.on_wait) if si is not None else []
                if len(waits) > max_waits:
                    lead = waits[: len(waits) - max_waits]
                    keep = waits[len(waits) - max_waits :]
                    for i in range(0, len(lead), max_waits):
                        nop = mybir.InstNoOp(
                            name=f"{inst.name}_w{i}", engine=inst.engine, ins=[], outs=[]
                        )
                        nop.sync_info = mybir.SyncInfo(
                            on_wait=lead[i : i + max_waits], on_update=[]
                        )
                        out.append(nop)
                    inst.sync_info = mybir.SyncInfo(
                        on_wait=keep, on_update=list(si.on_update)
                    )
                    changed = True
                out.append(inst)
            if changed:
                bb.instructions = out


def build_program(split_waits=True):
    _patch_tile_drain()
    nc = bass.Bass("TRN2", target_bir_lowering=False, debug=False, num_devices=N_CORES)

    xkv_d = nc.dram_tensor("xkv", [L, D], F32, kind="ExternalInput")
    wfc_d = nc.dram_tensor("wfc", [D, 3 * H * HD], F32, kind="ExternalInput")
    bfc_d = nc.dram_tensor("bfc", [3 * H * HD], F32, kind="ExternalInput")
    wfc2_d = nc.dram_tensor("wfc2", [H * HD, D], F32, kind="ExternalInput")
    bfc2_d = nc.dram_tensor("bfc2", [D], F32, kind="ExternalInput")
    ident_d = nc.dram_tensor("ident", [128, 128], F32, kind="ExternalInput")
    out_d = nc.dram_tensor("out", [WQ, D], F32, kind="ExternalOutput")

    NT = L // 128            # 16 token tiles
    ND = D // 128            # 8 d tiles
    NW = WQ // 128           # 8 query-token tiles
    NM = L // 128            # 16 key tiles

    with tile.TileContext(nc) as tc:
        pers = tc.alloc_tile_pool(name="pers", bufs=1)
        pmm = tc.alloc_tile_pool(name="pmm", bufs=2, space="PSUM")
        pu = tc.alloc_tile_pool(name="pu", bufs=2, space="PSUM")

        # --- constants ---
        ident = pers.tile([128, 128], BF16, tag="ident")
        nc.gpsimd.dma_start(ident[:, :], ident_d[:, :])
        ones = pers.tile([128, 128], BF16, tag="ones")
        nc.gpsimd.memset(ones[:, :], 1.0)
        eps = pers.tile([128, 1], F32, tag="eps")
        nc.gpsimd.memset(eps[:, :], LN_EPS)

        qkv_pool = tc.alloc_tile_pool(name="qkv", bufs=1)
        # q is stored zero-padded per head ([128,WQ] with only this head's 64
        # rows nonzero) so the scores matmul can use the full-K=128 kT pair as
        # stationary: K=64 stationaries with fresh weights cost ~2x (weight
        # load does not overlap the running matmul).
        qZ = [qkv_pool.tile([128, WQ], BF16, tag=f"qZ{h}", name=f"qZ{h}") for h in range(H)]
        kT = [qkv_pool.tile([128, L], BF16, tag=f"kT{i}", name=f"kT{i}") for i in range(ND)]
        vaug = [qkv_pool.tile([128, H * 65], BF16, tag=f"va{i}", name=f"va{i}") for i in range(NM)]
        # nv stored as head-pair tiles so out-proj accumulates with K=128
        nvP = [pers.tile([128, WQ], BF16, tag=f"nvp{e}", name=f"nvp{e}") for e in range(ND)]
        w2 = [pers.tile([128, D], BF16, tag=f"w2_{e}", name=f"w2_{e}") for e in range(ND)]
        b2 = pers.tile([1, D], BF16, tag="b2")

        # ---- phases 0-2 interleaved: transpose, v-proj, then per head-pair
        # q/k projection immediately followed by that pair's attention, so the
        # PE keeps dense work while ACT chews through the exps.
        with tc.tile_pool(name="ph12", bufs=1) as ph1:
            ph2 = ph1
            xkvT = [ph1.tile([128, L], BF16, tag=f"xkvT{i}", name=f"xkvT{i}") for i in range(ND)]

            wfc_r = wfc_d.rearrange("d (h c) -> d h c", c=3 * HD)
            bfc_r = bfc_d.rearrange("(h c) -> h c", c=3 * HD)

            # first x tiles in flight before the wv loads queue behind them
            xb_pre = []
            for ti in range(2):
                xb = ph1.tile([128, D], BF16, tag="xb", bufs=2, name=f"xbp{ti}")
                nc.gpsimd.dma_start(xb[:, :], xkv_d[ti * 128 : (ti + 1) * 128, :])
                xb_pre.append(xb)
            wvs = {}
            for c2 in range(2):
                for kd in range(ND):
                    w = ph1.tile([128, 512], BF16, tag=f"wv{c2}_{kd}", name=f"wv{c2}_{kd}")
                    nc.gpsimd.dma_start(
                        w[:, :],
                        wfc_r[kd * 128 : (kd + 1) * 128, c2 * 8 : (c2 + 1) * 8, 2 * HD : 3 * HD],
                    )
                    wvs[(c2, kd)] = w
            bv = ph1.tile([1, H * HD], BF16, tag="bv")
            nc.gpsimd.dma_start(bv[:, :], bfc_r[:, 2 * HD : 3 * HD])

            # per token-tile: transpose x, then immediately project v for that
            # key-tile (keeps ACT fed from the very start)
            for ti in range(NT):
                if ti < 2:
                    xb = xb_pre[ti]
                else:
                    xb = ph1.tile([128, D], BF16, tag="xb", bufs=2)
                    nc.gpsimd.dma_start(xb[:, :], xkv_d[ti * 128 : (ti + 1) * 128, :])
                for kd in range(ND):
                    pt = pmm.tile([128, 128], BF16, tag="mm")
                    nc.tensor.transpose(
                        pt[:, :], xb[:, kd * 128 : (kd + 1) * 128], ident[:, :]
                    )
                    nc.vector.tensor_copy(
                        xkvT[kd][:, ti * 128 : (ti + 1) * 128], pt[:, :]
                    )
                ps = pmm.tile([128, 1024], F32, tag="mm", name=f"vps{ti}")
                for c2 in range(2):
                    sl = slice(c2 * 512, (c2 + 1) * 512)
                    for kd in range(ND):
                        nc.tensor.matmul(
                            ps[:, sl],
                            xkvT[kd][:, ti * 128 : (ti + 1) * 128],
                            wvs[(c2, kd)][:, :],
                            start=(kd == 0),
                            stop=False,
                        )
                    nc.tensor.matmul(
                        ps[:, sl],
                        ones[0:1, 0:128],
                        bv[0:1, sl],
                        start=False,
                        stop=True,
                    )
                va = vaug[ti]
                va_r = va[:, :].rearrange("p (h c) -> p h c", c=65)
                nc.gpsimd.memset(va_r[:, :, 64:65], 1.0)
                nc.scalar.activation(
                    va_r[:, :, 0:64],
                    ps[:, :],
                    AF.Silu,
                )

            def project_qk(et):
                """q (et<ND) or k (et>=ND) projection for e-tile et%ND."""
                is_q = et < ND
                qi = et % ND
                c0 = 0 if is_q else HD
                if is_q:
                    nc.gpsimd.memset(qZ[2 * qi][:, :], 0.0)
                    nc.gpsimd.memset(qZ[2 * qi + 1][:, :], 0.0)
                wts = []
                for kd in range(ND):
                    w = ph1.tile([128, 128], BF16, tag="wqk", bufs=10, name=f"wqk{et}_{kd}")
                    nc.gpsimd.dma_start(
                        w[:, :],
                        wfc_r[kd * 128 : (kd + 1) * 128, 2 * qi : 2 * qi + 2, c0 : c0 + HD],
                    )
                    wts.append(w)
                bt = ph1.tile([128, 1], F32, tag="bqk", bufs=3, name=f"bqk{et}")
                nc.gpsimd.dma_start(bt[:, :], bfc_r[2 * qi : 2 * qi + 2, c0 : c0 + HD])
                ncols = WQ if is_q else L
                for half in range(ncols // 1024):
                    ps = pmm.tile([128, 1024], F32, tag="mm", name=f"qk{et}_{half}")
                    for tc2 in range(2):
                        t0 = half * 1024 + tc2 * 512
                        for kd in range(ND):
                            nc.tensor.matmul(
                                ps[:, tc2 * 512 : (tc2 + 1) * 512],
                                wts[kd][:, :],
                                xkvT[kd][:, t0 : t0 + 512],
                                start=(kd == 0),
                                stop=(kd == ND - 1),
                            )
                    if is_q:
                        for pi in range(2):
                            pr = pi * 64
                            nc.scalar.activation(
                                qZ[2 * qi + pi][pr : pr + 64, half * 1024 : (half + 1) * 1024],
                                ps[pr : pr + 64, :],
                                AF.Silu,
                                bias=bt[pr : pr + 64, :],
                            )
                    else:
                        nc.scalar.activation(
                            kT[qi][:, half * 1024 : (half + 1) * 1024],
                            ps[:, :],
                            AF.Silu,
                            bias=bt[:, :],
                        )

            def attn_mms(h):
                et = h // 2
                pr = (h % 2) * 64
                u = pu.tile([128, 1024], F32, tag="u", name=f"u{h}")
                # process key-tiles in pairs: both scores matmuls, both exps,
                # then both U matmuls — halves the stationary-shape transitions
                # on the PE (each scores->U switch costs ~160ns of weight-load)
                for mp in range(NM // 2):
                    exs = []
                    for mt in (2 * mp, 2 * mp + 1):
                        ps = pmm.tile([128, 1024], F32, tag="mm", name=f"sc{h}_{mt}")
                        for wc in range(2):
                            nc.tensor.matmul(
                                ps[:, wc * 512 : (wc + 1) * 512],
                                kT[et][:, mt * 128 : (mt + 1) * 128],
                                qZ[h][:, wc * 512 : (wc + 1) * 512],
                                start=True,
                                stop=True,
                            )
                        ex = ph2.tile([128, 1024], BF16, tag="exp", bufs=3, name=f"ex{h}_{mt}")
                        nc.scalar.activation(ex[:, :], ps[:, :], AF.Exp, scale=SCALE)
                        exs.append(ex)
                    for i, mt in enumerate((2 * mp, 2 * mp + 1)):
                        for wc in range(2):
                            sl = slice(wc * 512, (wc + 1) * 512)
                            nc.tensor.matmul(
                                u[0:65, sl],
                                vaug[mt][:, h * 65 : (h + 1) * 65],
                                exs[i][:, sl],
                                start=(mt == 0),
                                stop=(mt == NM - 1),
                            )
                return u

            def normalize(h, u):
                """Pipelined softmax-denominator normalization: issued one head
                late so the DVE reciprocal completes while the next head's
                matmuls keep the PE stream busy."""
                lnt = ph2.tile([128, 512], F32, tag="lnt", bufs=2, name=f"lnt{h}")
                rc = ph2.tile([128, 512], BF16, tag="recip", bufs=2, name=f"rc{h}")
                bcs = ph2.tile([64, 1024], BF16, tag="bcs", bufs=2, name=f"bcs{h}")
                bc = pmm.tile([128, 1024], F32, tag="mm", name=f"bc{h}")
                # 1/S = exp(-ln S) on ACT: shares the exp table (no DVE
                # reciprocal, which costs ~6.5us and blocks the DVE queue)
                for wc in range(2):
                    sl = slice(wc * 512, (wc + 1) * 512)
                    lsl = slice(0, 512)
                    nc.scalar.activation(lnt[64:65, lsl], u[64:65, sl], AF.Ln)
                    nc.scalar.activation(rc[64:65, lsl], lnt[64:65, lsl], AF.Exp, scale=-1.0)
                    nc.tensor.matmul(
                        bc[0:64, sl],
                        ones[64:65, 0:64],
                        rc[64:65, lsl],
                        start=True,
                        stop=True,
                    )
                nc.vector.tensor_copy(bcs[0:64, :], bc[0:64, :])
                if h % 2 == 0:
                    nc.vector.tensor_mul(nvP[h // 2][0:64, :], u[0:64, :], bcs[0:64, :])
                else:
                    nvt = ph2.tile([64, 1024], BF16, tag="nvt", bufs=2, name=f"nvt{h}")
                    nc.vector.tensor_mul(nvt[:, :], u[0:64, :], bcs[0:64, :])
                    nc.vector.stream_shuffle(nvP[h // 2][64:128, :], nvt[0:64, :], list(range(32)))

            # prefetch output-projection weights well before use
            for e in range(ND):
                nc.gpsimd.dma_start(w2[e][:, :], wfc2_d[e * 128 : (e + 1) * 128, :])
            nc.gpsimd.dma_start(b2[:, :], bfc2_d[:])

            project_qk(0)
            project_qk(ND)
            pending = None
            for et in range(ND):
                u_a = attn_mms(2 * et)
                if pending is not None:
                    normalize(*pending)
                if et + 1 < ND:
                    project_qk(et + 1)
                    project_qk(ND + et + 1)
                u_b = attn_mms(2 * et + 1)
                normalize(2 * et, u_a)
                pending = (2 * et + 1, u_b)
            normalize(*pending)

        # ---------------- phase 3: output projection + LN ------------------
        with tc.tile_pool(name="ph3", bufs=1) as ph3:
            for wt in range(NW):
                po = pmm.tile([128, 1024], F32, tag="mm")
                for dc in range(2):
                    sl = slice(dc * 512, (dc + 1) * 512)
                    for e in range(ND):
                        nc.tensor.matmul(
                            po[:, sl],
                            nvP[e][:, wt * 128 : (wt + 1) * 128],
                            w2[e][:, sl],
                            start=(e == 0),
                            stop=False,
                        )
                    nc.tensor.matmul(
                        po[:, sl],
                        ones[0:1, 0:128],
                        b2[0:1, sl],
                        start=False,
                        stop=True,
                    )
                msb = ph3.tile([128, 1024], F32, tag="m", bufs=2)
                nc.scalar.activation(msb[:, :], po[:, :], AF.Silu)
                xr = ph3.tile([128, 1024], F32, tag="xr", bufs=2)
                nc.gpsimd.dma_start(xr[:, :], xkv_d[wt * 128 : (wt + 1) * 128, :])
                y = ph3.tile([128, 1024], F32, tag="y", bufs=2)
                nc.vector.tensor_add(y[:, :], msb[:, :], xr[:, :])
                st = ph3.tile([128, 12], F32, tag="st", bufs=2)
                nc.vector.bn_stats(st[:, 0:6], y[:, 0:512])
                nc.vector.bn_stats(st[:, 6:12], y[:, 512:1024])
                mv = ph3.tile([128, 2], F32, tag="mv", bufs=2)
                nc.vector.bn_aggr(mv[:, :], st[:, :])
                sd = ph3.tile([128, 2], F32, tag="sd", bufs=2)
                nc.scalar.activation(sd[:, 0:1], mv[:, 1:2], AF.Sqrt, bias=eps[:, 0:1])
                nc.vector.reciprocal(sd[:, 1:2], sd[:, 0:1])
                ot = ph3.tile([128, 1024], F32, tag="ot", bufs=2)
                nc.vector.tensor_scalar(
                    ot[:, :],
                    y[:, :],
                    mv[:, 0:1],
                    sd[:, 1:2],
                    ALU.subtract,
                    ALU.mult,
                )
                nc.gpsimd.dma_start(out_d[wt * 128 : (wt + 1) * 128, :], ot[:, :])

        qkv_pool.release()
        pu.release()
        pmm.release()
        pers.release()

    if split_waits:
        _split_excess_waits(nc)
    return nc


_NC_CACHE = None


def _get_program():
    global _NC_CACHE
    if _NC_CACHE is None:
        _NC_CACHE = build_program()
    return _NC_CACHE


def make_in_maps(x, W_fc, b_fc, W_fc2, b_fc2):
    x = np.asarray(x, dtype=np.float32)
    W_fc = np.ascontiguousarray(np.asarray(W_fc, dtype=np.float32))
    b_fc = np.ascontiguousarray(np.asarray(b_fc, dtype=np.float32))
    W_fc2 = np.ascontiguousarray(np.asarray(W_fc2, dtype=np.float32))
    b_fc2 = np.ascontiguousarray(np.asarray(b_fc2, dtype=np.float32))
    ident = np.eye(128, dtype=np.float32)
    in_maps = []
    for i in range(N_CORES):
        b = i // 2
        w0 = (i % 2) * WQ
        xkv = np.ascontiguousarray(
            np.concatenate([x[b, w0:], x[b, :w0]], axis=0)
        )
        in_maps.append(
            {
                "xkv": xkv,
                "wfc": W_fc,
                "bfc": b_fc,
                "wfc2": W_fc2,
                "bfc2": b_fc2,
                "ident": ident,
            }
        )
    return in_maps


def kernel(x, W_fc, b_fc, W_fc2, b_fc2, **extra):
    nc = _get_program()
    in_maps = make_in_maps(x, W_fc, b_fc, W_fc2, b_fc2)
    res = run_bass_kernel_spmd(nc, in_maps, list(range(N_CORES)))
    out = np.empty((B, L, D), dtype=np.float32)
    for i in range(N_CORES):
        b = i // 2
        w0 = (i % 2) * WQ
        out[b, w0 : w0 + WQ] = res.results[i]["out"]
    return out
